# revision 1
# baseline (speedup 1.0000x reference)
"""CRF loss kernel for Trainium2 (8 NeuronCores, data-parallel over batch).

Strategy (per core, batch shard of 64 rows = 32768 positions):
  - emissions gather sum_{b,s} m*E[b,s,tags] via one-hot matmuls on PE:
    E is split exactly as E = bf16(E) + bf16(E - bf16(E)) (17-18 mantissa
    bits kept); both halves go through full-rate bf16 matmuls against a
    bf16 one-hot of the (mask-folded) tags, accumulating in fp32 PSUM.
    Diagonal of the accumulated [T,T] PSUM = emission score.
  - transition score via pair co-occurrence counts C = Hprev^T @ Hcur
    (bf16 one-hots, exact 0/1 counts in fp32 PSUM), then sum(C * T).
  - mask folding: tag + 128*(1-m) pushes masked positions out of iota
    range so their one-hot row is all zero.
  - the two scalar partial sums and the mask count are reduced on-chip
    to a [1,8] vector per core; the 8-way combine + division is host-side.
"""
import sys
import json

for p in ('/opt/trn_rl_repo', '/opt/trn_rl_repo/concourse'):
    if p not in sys.path:
        sys.path.insert(0, p)

import numpy as np

B, S, T = 512, 512, 128
NCORES = 8
BSH = B // NCORES              # 64 batch rows per core
NPOS = BSH * S                 # 32768 positions per core
NTILE = NPOS // 128            # 256 tag-tiles of 128 positions
NBLK = NTILE // 4              # 64 blocks of [128, 4, 128]
# fraction of lo-subtract blocks on DVE (rest on GPSIMD)
LO_DVE_MOD = 3                 # g % LO_DVE_MOD == 0 -> DVE


def _split_waits_json(bir_bytes: bytes, max_waits: int = 1) -> bytes:
    """This walrus build accepts at most ONE sync-wait per instruction;
    hoist extra waits onto single-wait NoOps inserted before the inst."""
    d = json.loads(bir_bytes)
    ctr = 0
    for f in d['functions']:
        for blk in f['blocks']:
            insts = blk.get('instructions')
            if not insts:
                continue
            out = []
            changed = False
            for ins in insts:
                si = ins.get('sync_info')
                if si and len(si.get('on_wait') or []) > max_waits:
                    waits = si['on_wait']
                    for w in waits[:-max_waits]:
                        ctr += 1
                        nop = {'engine': ins['engine'], 'ins': [], 'outs': [],
                               'name': f'wsplit-{ctr}', 'opcode': 'NoOp',
                               'sync_info': {'on_wait': [w], 'on_update': []}}
                        if 'debug' in ins:
                            nop['debug'] = ins['debug']
                        out.append(nop)
                    si['on_wait'] = waits[-max_waits:]
                    changed = True
                out.append(ins)
            if changed:
                blk['instructions'] = out
    return json.dumps(d).encode()


_patched = False


def _install_patch(bass_module):
    global _patched
    if _patched:
        return
    _patched = True
    orig = bass_module.Bass.to_json_bytes

    def patched(self):
        return _split_waits_json(orig(self))

    bass_module.Bass.to_json_bytes = patched


def _build():
    import concourse.bass as bass
    import concourse.mybir as mybir
    import concourse.tile as tile
    from concourse.masks import make_identity
    _install_patch(bass)
    f32 = mybir.dt.float32
    bf16 = mybir.dt.bfloat16
    u16 = mybir.dt.uint16
    i32 = mybir.dt.int32
    Alu = mybir.AluOpType

    nc = bass.Bass()
    em = nc.dram_tensor('em', [NPOS, T], f32, kind='ExternalInput')
    tg = nc.dram_tensor('tg', [NPOS + 1], u16, kind='ExternalInput')
    mk = nc.dram_tensor('mk', [NPOS + 1], u16, kind='ExternalInput')
    tr = nc.dram_tensor('tr', [T, T], f32, kind='ExternalInput')
    out = nc.dram_tensor('out', [1, 8], f32, kind='ExternalOutput')

    with tile.TileContext(nc) as tc:
        with tc.tile_pool(name='per', bufs=1) as per, \
             tc.tile_pool(name='eblk', bufs=4) as eblk, \
             tc.tile_pool(name='hblk', bufs=4) as hblk, \
             tc.tile_pool(name='ps', bufs=1, space='PSUM') as psp:

            # ---- constants ----
            iota_i = per.tile([128, 128], i32)
            nc.gpsimd.iota(iota_i, pattern=[[1, 128]], base=0, channel_multiplier=0)
            iota_b = per.tile([128, 128], bf16)
            nc.vector.tensor_copy(iota_b, iota_i)
            ident = per.tile([128, 128], f32)
            make_identity(nc, ident)
            ones_col = per.tile([128, 1], f32)
            nc.vector.memset(ones_col, 1.0)
            t_sb = per.tile([128, 128], f32)
            nc.sync.dma_start(out=t_sb, in_=tr[:, :])

            # ---- tags / mask (transposed to [pos%128, tile] layout) ----
            tg_cur = per.tile([128, NTILE], u16)
            nc.sync.dma_start_transpose(tg_cur, tg[1:NPOS + 1].rearrange("(a b) -> a b", b=128))
            tg_prev = per.tile([128, NTILE], u16)
            nc.sync.dma_start_transpose(tg_prev, tg[0:NPOS].rearrange("(a b) -> a b", b=128))
            mk_cur = per.tile([128, NTILE], u16)
            nc.sync.dma_start_transpose(mk_cur, mk[1:NPOS + 1].rearrange("(a b) -> a b", b=128))
            mk_prev = per.tile([128, NTILE], u16)
            nc.sync.dma_start_transpose(mk_prev, mk[0:NPOS].rearrange("(a b) -> a b", b=128))

            tgc_f = per.tile([128, NTILE], f32)
            nc.vector.tensor_copy(tgc_f, tg_cur)
            tgp_f = per.tile([128, NTILE], f32)
            nc.vector.tensor_copy(tgp_f, tg_prev)
            mc_f = per.tile([128, NTILE], f32)
            nc.vector.tensor_copy(mc_f, mk_cur)
            mp_f = per.tile([128, NTILE], f32)
            nc.vector.tensor_copy(mp_f, mk_prev)

            # masked cur tags: tg + 128 - 128*m
            tmp = per.tile([128, NTILE], f32)
            nc.vector.tensor_scalar(out=tmp, in0=mc_f, scalar1=-128.0, scalar2=128.0,
                                    op0=Alu.mult, op1=Alu.add)
            mtag_c = per.tile([128, NTILE], f32)
            nc.vector.tensor_add(mtag_c, tgc_f, tmp)

            # pair mask pm = m_cur * m_prev, zeroed at batch-row starts
            pm = per.tile([128, NTILE], f32)
            nc.vector.tensor_mul(pm, mc_f, mp_f)
            rs_i = per.tile([128, NTILE], i32)   # p + 128*(tile%4); ==0 at row starts
            nc.gpsimd.iota(rs_i, pattern=[[0, NTILE // 4], [128, 4]], base=0,
                           channel_multiplier=1)
            rs_f = per.tile([128, NTILE], f32)
            nc.vector.tensor_copy(rs_f, rs_i)
            rs_m = per.tile([128, NTILE], f32)
            nc.vector.tensor_scalar(out=rs_m, in0=rs_f, scalar1=0.0, scalar2=None,
                                    op0=Alu.not_equal)
            nc.vector.tensor_mul(pm, pm, rs_m)

            # masked prev tags: tg_prev + 128 - 128*pm
            nc.vector.tensor_scalar(out=tmp, in0=pm, scalar1=-128.0, scalar2=128.0,
                                    op0=Alu.mult, op1=Alu.add)
            mtag_p = per.tile([128, NTILE], f32)
            nc.vector.tensor_add(mtag_p, tgp_f, tmp)

            # ---- accumulators ----
            ps_emit = psp.tile([128, 256], f32)
            ps_c = psp.tile([128, 128], f32)

            em_r = em.rearrange("(g j p) t -> g p j t", p=128, j=4)

            for g in range(NBLK):
                e_blk = eblk.tile([128, 4, 128], f32, tag='e')
                nc.sync.dma_start(out=e_blk, in_=em_r[g])
                hl_blk = eblk.tile([128, 4, 256], bf16, tag='hl')
                hi_blk = hl_blk[:, :, 0:128]
                lo_blk = hl_blk[:, :, 128:256]
                nc.scalar.activation(out=hi_blk, in_=e_blk,
                                     func=mybir.ActivationFunctionType.Copy)
                if g % LO_DVE_MOD == 0:
                    nc.vector.tensor_sub(lo_blk, e_blk, hi_blk)
                else:
                    nc.gpsimd.tensor_sub(lo_blk, e_blk, hi_blk)
                hm = hblk.tile([128, 4, 128], bf16, tag='hm')
                hp = hblk.tile([128, 4, 128], bf16, tag='hp')
                for j in range(4):
                    k = 4 * g + j
                    nc.vector.tensor_scalar(out=hm[:, j, :], in0=iota_b,
                                            scalar1=mtag_c[:, k:k + 1], scalar2=None,
                                            op0=Alu.is_equal)
                    nc.vector.tensor_scalar(out=hp[:, j, :], in0=iota_b,
                                            scalar1=mtag_p[:, k:k + 1], scalar2=None,
                                            op0=Alu.is_equal)
                for j in range(4):
                    first = (g == 0 and j == 0)
                    last = (g == NBLK - 1 and j == 3)
                    nc.tensor.matmul(ps_emit, lhsT=hm[:, j, :], rhs=hl_blk[:, j, :],
                                     start=first, stop=last, skip_group_check=True)
                    nc.tensor.matmul(ps_c, lhsT=hp[:, j, :], rhs=hm[:, j, :],
                                     start=first, stop=last, skip_group_check=True)

            # ---- final reductions ----
            red = per.tile([128, 8], f32)
            nc.vector.memset(red, 0.0)
            scr = per.tile([128, 256], f32)
            nc.vector.tensor_mul(scr[:, 0:128], ps_emit[:, 0:128], ident)
            nc.vector.tensor_mul(scr[:, 128:256], ps_emit[:, 128:256], ident)
            nc.vector.tensor_reduce(out=red[:, 0:1], in_=scr,
                                    axis=mybir.AxisListType.X, op=Alu.add)
            scr2 = per.tile([128, 128], f32)
            nc.vector.tensor_mul(scr2, ps_c, t_sb)
            nc.vector.tensor_reduce(out=red[:, 1:2], in_=scr2,
                                    axis=mybir.AxisListType.X, op=Alu.add)
            nc.vector.tensor_reduce(out=red[:, 2:3], in_=mc_f,
                                    axis=mybir.AxisListType.X, op=Alu.add)
            ps_fin = psp.tile([1, 8], f32)
            nc.tensor.matmul(ps_fin, lhsT=ones_col, rhs=red, start=True, stop=True,
                             skip_group_check=True)
            fin = per.tile([1, 8], f32)
            nc.vector.tensor_copy(fin, ps_fin)
            nc.sync.dma_start(out=out[:, :], in_=fin)

    return nc


_nc_cache = None
last_results = None


def kernel(emissions, tags, mask, transitions, _trace=False):
    global _nc_cache, last_results
    from concourse.bass_utils import run_bass_kernel_spmd
    if _nc_cache is None:
        _nc_cache = _build()
    nc = _nc_cache

    em_flat = np.ascontiguousarray(emissions.reshape(B * S, T).astype(np.float32, copy=False))
    tg_flat = tags.reshape(-1).astype(np.uint16)
    mk_flat = mask.reshape(-1).astype(np.uint16)
    trf = np.ascontiguousarray(transitions.astype(np.float32, copy=False))

    in_maps = []
    for c in range(NCORES):
        lo, hi = c * NPOS, (c + 1) * NPOS
        tg_pad = np.zeros(NPOS + 1, dtype=np.uint16)
        tg_pad[1:] = tg_flat[lo:hi]
        mk_pad = np.zeros(NPOS + 1, dtype=np.uint16)
        mk_pad[1:] = mk_flat[lo:hi]
        in_maps.append({'em': np.ascontiguousarray(em_flat[lo:hi]),
                        'tg': tg_pad, 'mk': mk_pad, 'tr': trf})

    res = run_bass_kernel_spmd(nc, in_maps, core_ids=list(range(NCORES)),
                               trace=_trace)
    last_results = res
    emit = trans = cnt = 0.0
    for r in res.results:
        v = r['out'][0]
        emit += float(v[0])
        trans += float(v[1])
        cnt += float(v[2])
    return np.float32((emit + trans) / cnt)



# revision 12
# speedup vs baseline: 2.8176x; 2.8176x over previous
"""CRF loss kernel for Trainium2 (8 NeuronCores, data-parallel over batch).

v4 = v3 + scheduling fixes:
  - all small inputs arrive in TWO consolidated DMAs (one int16 table,
    one fp32 table) issued before the gathers, so the one-hot builds and
    transition IndirectCopies never stall the Pool/DVE pipelines.
  - IndirectCopies split into 4x320 and slotted between early gather
    descriptor preps (each fits the prep/transfer slack: no DMA bubble).
  - transition multiply+reduce runs mid-stream, off the tail.
  - per-core output is [128,8] partials; host sums partitions & cores.

Core strategy (per core = 64 batch rows, 32768 positions):
  - emissions host-cast to bf16; maximal mask=1 runs fetched by SWDGE
    dma_gather as pairs (512B, line rate) + one single (256B) per odd
    run, via three streams (even pairs / odd pairs / singles).
  - em[pos, tag] selected by one-hot diag-matmul on PE (bf16, fp32
    accum); ones-column matmuls accumulate the mask count.
  - transitions: pm=1 pairs bucketed by tp//16; IndirectCopy reads
    T[p, tc_j] on all 16 partitions of the bucket's gpsimd core; a 0/1
    selector mask keeps partition tp_j; multiply+reduce on DVE.
"""
import sys
import json

for p in ('/opt/trn_rl_repo', '/opt/trn_rl_repo/concourse'):
    if p not in sys.path:
        sys.path.insert(0, p)

import numpy as np
import ml_dtypes

B, S, T = 512, 512, 128
NCORES = 8
BSH = B // NCORES              # 64 batch rows per core
NPOS = BSH * S                 # 32768 positions per core

PEG = 25                       # even-start pair groups (25*128=3200 slots)
POG = 25                       # odd-start pair groups
SG = 47                        # single groups (47*128=6016 slots)
# chunks capped at 7 groups (896 descriptors): >=1664 descriptors in one
# SWDGE gather overflows the descriptor scratch ring and wedges the device
PE_CH = [7, 7, 7, 4]
PO_CH = [7, 7, 7, 4]
S_CH = [7, 7, 7, 7, 7, 7, 3, 2]
NT = 1280                      # padded transition pairs per gpsimd core
NTCOL = NT // 16
NIC = 4                        # transitions IndirectCopy split
# int16 table layout: [pidx | oidx | sidx | tidx]
I_PE, I_PO, I_S, I_T = 0, PEG * 8, (PEG + POG) * 8, (PEG + POG + SG) * 8
ICOLS = I_T + NTCOL
# fp32 table layout: [mp1 | mp2 | msng | stm | tr | iota | ident]
F_MP1 = 0
F_MP2 = F_MP1 + PEG + POG
F_MS = F_MP2 + PEG + POG
F_STM = F_MS + SG
F_TR = F_STM + NT
F_IO = F_TR + 128
F_ID = F_IO + 128
FCOLS = F_ID + 128


def _split_waits_json(bir_bytes: bytes, max_waits: int = 1) -> bytes:
    """This walrus build accepts at most ONE sync-wait per instruction;
    hoist extra waits onto single-wait NoOps inserted before the inst."""
    d = json.loads(bir_bytes)
    ctr = 0
    for f in d['functions']:
        for blk in f['blocks']:
            insts = blk.get('instructions')
            if not insts:
                continue
            out = []
            changed = False
            for ins in insts:
                si = ins.get('sync_info')
                if si and len(si.get('on_wait') or []) > max_waits:
                    waits = si['on_wait']
                    for w in waits[:-max_waits]:
                        ctr += 1
                        nop = {'engine': ins['engine'], 'ins': [], 'outs': [],
                               'name': f'wsplit-{ctr}', 'opcode': 'NoOp',
                               'sync_info': {'on_wait': [w], 'on_update': []}}
                        if 'debug' in ins:
                            nop['debug'] = ins['debug']
                        out.append(nop)
                    si['on_wait'] = waits[-max_waits:]
                    changed = True
                out.append(ins)
            if changed:
                blk['instructions'] = out
    return json.dumps(d).encode()


_patched = False


def _install_patch(bass_module):
    global _patched
    if _patched:
        return
    _patched = True
    orig = bass_module.Bass.to_json_bytes

    def patched(self):
        return _split_waits_json(orig(self))

    bass_module.Bass.to_json_bytes = patched


def _build():
    import concourse.bass as bass
    import concourse.mybir as mybir
    import concourse.tile as tile
    from concourse import library_config
    _install_patch(bass)
    f32 = mybir.dt.float32
    bf16 = mybir.dt.bfloat16
    i16 = mybir.dt.int16
    Alu = mybir.AluOpType

    nc = bass.Bass()
    emp = nc.dram_tensor('emp', [NPOS // 2, 256], bf16, kind='ExternalInput')
    emo = nc.dram_tensor('emo', [NPOS // 2 - 1, 256], bf16, kind='ExternalInput')
    ems = nc.dram_tensor('ems', [NPOS, 128], bf16, kind='ExternalInput')
    itab = nc.dram_tensor('itab', [128, ICOLS], i16, kind='ExternalInput')
    ftab = nc.dram_tensor('ftab', [128, FCOLS], f32, kind='ExternalInput')
    out = nc.dram_tensor('out', [128, 8], f32, kind='ExternalOutput')

    with tile.TileContext(nc) as tc:
        with tc.tile_pool(name='per', bufs=1) as per, \
             tc.tile_pool(name='pe_p', bufs=4) as pe_p, \
             tc.tile_pool(name='po_p', bufs=4) as po_p, \
             tc.tile_pool(name='s_p', bufs=8) as s_p, \
             tc.tile_pool(name='ps', bufs=1, space='PSUM') as psp:

            # ---- two consolidated input DMAs (first gather's idx cols
            # split off so its SWDGE prep starts as early as possible) ----
            it_sb = per.tile([128, ICOLS], i16)
            c00 = PE_CH[0] * 8
            nc.sync.dma_start(out=it_sb[:, 0:c00], in_=itab[:, 0:c00])
            nc.sync.dma_start(out=it_sb[:, c00:ICOLS], in_=itab[:, c00:ICOLS])
            ft_sb = per.tile([128, FCOLS], f32)
            nc.sync.dma_start(out=ft_sb, in_=ftab[:, :])
            nc.gpsimd.load_library(library_config.mlp)

            iota_b = per.tile([128, 128], bf16)
            nc.vector.tensor_copy(iota_b, ft_sb[:, F_IO:F_IO + 128])
            ones_b = per.tile([128, 1], bf16)
            nc.vector.memset(ones_b, 1.0)

            # ---- one-hot tiles ----
            hpair = per.tile([128, PEG + POG, 256], bf16)
            for g in range(PEG + POG):
                nc.vector.tensor_scalar(out=hpair[:, g, 0:128], in0=iota_b,
                                        scalar1=ft_sb[:, F_MP1 + g:F_MP1 + g + 1],
                                        scalar2=None, op0=Alu.is_equal)
                nc.vector.tensor_scalar(out=hpair[:, g, 128:256], in0=iota_b,
                                        scalar1=ft_sb[:, F_MP2 + g:F_MP2 + g + 1],
                                        scalar2=None, op0=Alu.is_equal)
            hsng = per.tile([128, SG, 128], bf16)
            for g in range(SG):
                nc.vector.tensor_scalar(out=hsng[:, g, :], in0=iota_b,
                                        scalar1=ft_sb[:, F_MS + g:F_MS + g + 1],
                                        scalar2=None, op0=Alu.is_equal)

            # ---- gathers + transitions ICs + matmul accumulation ----
            ps_emit = psp.tile([128, 128], f32)
            ps_cnt = psp.tile([128, 1], f32)
            tg_out = per.tile([128, NT], f32)

            streams = [
                (emp, I_PE, PE_CH, 256, 0, pe_p),
                (emo, I_PO, PO_CH, 256, PEG, po_p),
                (ems, I_S, S_CH, 128, 0, s_p),
            ]
            plan = []
            for dram, ibase, chunks, width, gbase, pool in streams:
                g0 = 0
                for ng in chunks:
                    plan.append((dram, ibase, g0, ng, width, gbase, pool))
                    g0 += ng

            n_mm = sum(2 * ng if w == 256 else ng
                       for _, _, _, ng, w, _, _ in plan)
            mm_i = 0
            nh = NT // NIC
            n_pair_ch = len(PE_CH) + len(PO_CH)
            for ci, (dram, ibase, g0, ng, width, gbase, pool) in enumerate(plan):
                nidx = ng * 128
                gt = pool.tile([128, ng, width], bf16, tag='g')
                nc.gpsimd.dma_gather(
                    out_ap=gt, in_ap=dram[:, :],
                    idxs_ap=it_sb[:, ibase + g0 * 8:ibase + (g0 + ng) * 8],
                    num_idxs=nidx, num_idxs_reg=nidx, elem_size=width)
                # transitions ICs fill the Pool buffer-wait gaps during the
                # singles chunks; done well before the tail
                if n_pair_ch <= ci < n_pair_ch + NIC:
                    k = ci - n_pair_ch
                    c0 = I_T + k * nh // 16
                    c1 = I_T + (k + 1) * nh // 16
                    nc.gpsimd.indirect_copy(
                        tg_out[:, k * nh:(k + 1) * nh],
                        ft_sb[:, F_TR:F_TR + 128],
                        it_sb[:, c0:c1].bitcast(mybir.dt.uint16), True)

                for c in range(ng):
                    gg = gbase + g0 + c
                    if width == 256:
                        halves = ((hpair[:, gg, 0:128], gt[:, c, 0:128]),
                                  (hpair[:, gg, 128:256], gt[:, c, 128:256]))
                    else:
                        halves = ((hsng[:, gg, :], gt[:, c, :]),)
                    for h, rhs in halves:
                        nc.tensor.matmul(ps_emit, lhsT=h, rhs=rhs,
                                         start=(mm_i == 0), stop=(mm_i == n_mm - 1),
                                         skip_group_check=True)
                        nc.tensor.matmul(ps_cnt, lhsT=h, rhs=ones_b,
                                         start=(mm_i == 0), stop=(mm_i == n_mm - 1),
                                         skip_group_check=True)
                        mm_i += 1
            tmul = per.tile([128, NT], f32)
            nc.vector.tensor_mul(tmul, tg_out, ft_sb[:, F_STM:F_STM + NT])
            red = per.tile([128, 8], f32)
            nc.vector.memset(red, 0.0)
            nc.vector.tensor_reduce(out=red[:, 1:2], in_=tmul,
                                    axis=mybir.AxisListType.X, op=Alu.add)

            # ---- final: diagonal + count -> [128,8] partials ----
            scr = per.tile([128, 128], f32)
            nc.vector.tensor_mul(scr, ps_emit, ft_sb[:, F_ID:F_ID + 128])
            nc.vector.tensor_reduce(out=red[:, 0:1], in_=scr,
                                    axis=mybir.AxisListType.X, op=Alu.add)
            nc.vector.tensor_copy(red[:, 2:3], ps_cnt)
            nc.sync.dma_start(out=out[:, :], in_=red)

    from concourse.library_overlay import lower_extended_insts
    lower_extended_insts(nc)
    return nc


def _wrap16(flat, cols):
    """slot i -> [i%16, i//16], replicated across the 8 gpsimd cores."""
    return np.tile(flat.reshape(cols, 16).T, (8, 1))


def _prep_core(em_bf, tg_flat, mk_flat, lo):
    """Host-side index prep for one core's shard [lo, lo+NPOS)."""
    tg = tg_flat[lo:lo + NPOS]
    mk = mk_flat[lo:lo + NPOS]
    m = mk.astype(np.int8)

    # ---- run decomposition: singles (odd runs) + pairs ----
    prev = np.empty_like(m)
    prev[0] = 0
    prev[1:] = m[:-1]
    nxt = np.empty_like(m)
    nxt[-1] = 0
    nxt[:-1] = m[1:]
    starts = np.nonzero(m & (1 - prev))[0]
    ends = np.nonzero(m & (1 - nxt))[0]
    lens = ends - starts + 1
    singles = starts[(lens & 1) == 1]
    pbase = starts + (lens & 1)
    npair = lens >> 1
    tot = int(npair.sum())
    run_off = np.zeros(len(starts), dtype=np.int64)
    np.cumsum(npair[:-1], out=run_off[1:])
    k = np.arange(tot) - np.repeat(run_off, npair)
    pstart = np.repeat(pbase, npair) + 2 * k
    pe_s = pstart[(pstart & 1) == 0]
    po_s = pstart[(pstart & 1) == 1]

    def fill(slots_pos, cap, idx_of):
        n = slots_pos.size
        assert n <= cap * 128, f"{n} > {cap * 128}"
        idx = np.zeros(cap * 128, dtype=np.int16)
        idx[:n] = idx_of(slots_pos).astype(np.int16)
        return idx

    pe_idx = fill(pe_s, PEG, lambda s: s >> 1)
    po_idx = fill(po_s, POG, lambda s: (s - 1) >> 1)
    s_idx = fill(singles, SG, lambda s: s)

    def tagtab(slots_pos, cap, toff):
        t = np.full(cap * 128, 1000.0, dtype=np.float32)
        t[:slots_pos.size] = tg[slots_pos + toff].astype(np.float32)
        return t.reshape(cap, 128).T  # [p, g]

    # ---- transition pairs: within-row (s>=1), both masked ----
    mk2 = mk.reshape(BSH, S)
    tg2 = tg.reshape(BSH, S)
    pm = mk2[:, 1:] & mk2[:, :-1]
    tp = tg2[:, :-1][pm].astype(np.int64)
    tc = tg2[:, 1:][pm].astype(np.int64)
    tidx = np.zeros((128, NTCOL), dtype=np.int16)
    stm = np.zeros((128, NT), dtype=np.float32)
    for g in range(8):
        sel = (tp >> 4) == g
        tpg = tp[sel]
        tcg = tc[sel]
        cnt = tpg.size
        assert cnt <= NT, f"bucket {g} count {cnt} exceeds NT {NT}"
        buf = np.zeros(NT, dtype=np.uint16)
        buf[:cnt] = tcg.astype(np.uint16)
        tidx[16 * g:16 * (g + 1), :] = buf.reshape(NTCOL, 16).T.view(np.int16)
        j = np.arange(cnt)
        stm[tpg, j] = 1.0

    itabv = np.concatenate([
        _wrap16(pe_idx, PEG * 8), _wrap16(po_idx, POG * 8),
        _wrap16(s_idx, SG * 8), tidx], axis=1)
    ftabv = np.concatenate([
        tagtab(pe_s, PEG, 0), tagtab(po_s, POG, 0),
        tagtab(pe_s, PEG, 1), tagtab(po_s, POG, 1),
        tagtab(singles, SG, 0), stm,
        np.zeros((128, 128), np.float32),  # tr placeholder, filled by caller
        np.tile(np.arange(128, dtype=np.float32), (128, 1)),
        np.eye(128, dtype=np.float32)], axis=1)

    emc = em_bf[lo:lo + NPOS]
    flat = emc.reshape(-1)
    return {'emp': flat.reshape(NPOS // 2, 256),
            'emo': np.ascontiguousarray(
                flat[128:NPOS * 128 - 128].reshape(NPOS // 2 - 1, 256)),
            'ems': emc,
            'itab': np.ascontiguousarray(itabv),
            'ftab': np.ascontiguousarray(ftabv)}


_nc_cache = None
last_results = None


def kernel(emissions, tags, mask, transitions, _trace=False):
    global _nc_cache, last_results
    from concourse.bass_utils import run_bass_kernel_spmd
    if _nc_cache is None:
        _nc_cache = _build()
    nc = _nc_cache

    em_bf = np.ascontiguousarray(
        emissions.reshape(B * S, T).astype(ml_dtypes.bfloat16))
    tg_flat = tags.reshape(-1).astype(np.int64)
    mk_flat = mask.reshape(-1).astype(bool)
    trf = transitions.astype(np.float32)

    in_maps = []
    for c in range(NCORES):
        mmap = _prep_core(em_bf, tg_flat, mk_flat, c * NPOS)
        mmap['ftab'][:, F_TR:F_TR + 128] = trf
        in_maps.append(mmap)

    res = run_bass_kernel_spmd(nc, in_maps, core_ids=list(range(NCORES)),
                               trace=_trace)
    last_results = res
    emit = trans = cnt = 0.0
    for r in res.results:
        v = r['out']  # [128, 8] partials
        emit += float(v[:, 0].sum())
        trans += float(v[:, 1].sum())
        cnt += float(v[:, 2].sum())
    return np.float32((emit + trans) / cnt)


# revision 13
# speedup vs baseline: 3.1097x; 1.1037x over previous
"""CRF loss kernel for Trainium2 (8 NeuronCores, data-parallel over batch).

v4 = v3 + scheduling fixes:
  - all small inputs arrive in TWO consolidated DMAs (one int16 table,
    one fp32 table) issued before the gathers, so the one-hot builds and
    transition IndirectCopies never stall the Pool/DVE pipelines.
  - IndirectCopies split into 4x320 and slotted between early gather
    descriptor preps (each fits the prep/transfer slack: no DMA bubble).
  - transition multiply+reduce runs mid-stream, off the tail.
  - per-core output is [128,8] partials; host sums partitions & cores.

Core strategy (per core = 64 batch rows, 32768 positions):
  - emissions host-cast to bf16; maximal mask=1 runs fetched by SWDGE
    dma_gather as pairs (512B, line rate) + one single (256B) per odd
    run, via three streams (even pairs / odd pairs / singles).
  - em[pos, tag] selected by one-hot diag-matmul on PE (bf16, fp32
    accum); ones-column matmuls accumulate the mask count.
  - transitions: pm=1 pairs bucketed by tp//16; IndirectCopy reads
    T[p, tc_j] on all 16 partitions of the bucket's gpsimd core; a 0/1
    selector mask keeps partition tp_j; multiply+reduce on DVE.
"""
import sys
import json

for p in ('/opt/trn_rl_repo', '/opt/trn_rl_repo/concourse'):
    if p not in sys.path:
        sys.path.insert(0, p)

import numpy as np
import ml_dtypes

B, S, T = 512, 512, 128
NCORES = 8
BSH = B // NCORES              # 64 batch rows per core
NPOS = BSH * S                 # 32768 positions per core

# v5: singles are merged into the pair streams (the masked partner row is
# gathered too; its one-hot row is zero), so every descriptor is a 512B
# line-rate transfer and there are only two gather streams.
PEG = 46                       # even-start pair-slot groups (46*128=5888)
POG = 46                       # odd-start pair-slot groups
# chunks capped at 7 groups (896 descriptors): >=1280 descriptors in one
# SWDGE gather overflows the 1024-desc scratch ring and wedges the device
PE_CH = [7, 7, 7, 7, 7, 7, 4]
PO_CH = [7, 7, 7, 7, 7, 7, 4]
NT = 1280                      # padded transition pairs per gpsimd core
NTCOL = NT // 16
NIC = 2                        # transitions IndirectCopy split (<=1024 dst)
# int16 table layout: [pidx | oidx | tidx]
I_PE, I_PO, I_T = 0, PEG * 8, (PEG + POG) * 8
ICOLS = I_T + NTCOL
PSPLIT = 23                    # groups 0..22 per class hold real pairs;
                               # groups 23..45 hold singles (t2 half skipped)
# fp32 table layout: [mp1 | mp2 | iota | tr | ident]
F_MP1 = 0
F_MP2 = F_MP1 + PEG + POG
F_IO = F_MP2 + PEG + POG
F_TR = F_IO + 128
F_ID = F_TR + 128
FCOLS = F_ID + 128


def _split_waits_json(bir_bytes: bytes, max_waits: int = 1) -> bytes:
    """This walrus build accepts at most ONE sync-wait per instruction;
    hoist extra waits onto single-wait NoOps inserted before the inst."""
    d = json.loads(bir_bytes)
    ctr = 0
    for f in d['functions']:
        for blk in f['blocks']:
            insts = blk.get('instructions')
            if not insts:
                continue
            out = []
            changed = False
            for ins in insts:
                si = ins.get('sync_info')
                if si and len(si.get('on_wait') or []) > max_waits:
                    waits = si['on_wait']
                    for w in waits[:-max_waits]:
                        ctr += 1
                        nop = {'engine': ins['engine'], 'ins': [], 'outs': [],
                               'name': f'wsplit-{ctr}', 'opcode': 'NoOp',
                               'sync_info': {'on_wait': [w], 'on_update': []}}
                        if 'debug' in ins:
                            nop['debug'] = ins['debug']
                        out.append(nop)
                    si['on_wait'] = waits[-max_waits:]
                    changed = True
                out.append(ins)
            if changed:
                blk['instructions'] = out
    return json.dumps(d).encode()


_patched = False


def _install_patch(bass_module):
    global _patched
    if _patched:
        return
    _patched = True
    orig = bass_module.Bass.to_json_bytes

    def patched(self):
        return _split_waits_json(orig(self))

    bass_module.Bass.to_json_bytes = patched


def _build():
    import concourse.bass as bass
    import concourse.mybir as mybir
    import concourse.tile as tile
    from concourse import library_config
    _install_patch(bass)
    f32 = mybir.dt.float32
    bf16 = mybir.dt.bfloat16
    i16 = mybir.dt.int16
    Alu = mybir.AluOpType

    nc = bass.Bass()
    emp = nc.dram_tensor('emp', [NPOS // 2, 256], bf16, kind='ExternalInput')
    emo = nc.dram_tensor('emo', [NPOS // 2 - 1, 256], bf16, kind='ExternalInput')
    itab = nc.dram_tensor('itab', [128, ICOLS], i16, kind='ExternalInput')
    ftab = nc.dram_tensor('ftab', [128, FCOLS], f32, kind='ExternalInput')
    stmb = nc.dram_tensor('stmb', [128, NT], bf16, kind='ExternalInput')
    out = nc.dram_tensor('out', [128, 8], f32, kind='ExternalOutput')

    with tile.TileContext(nc) as tc:
        with tc.tile_pool(name='per', bufs=1) as per, \
             tc.tile_pool(name='pe_p', bufs=5) as pe_p, \
             tc.tile_pool(name='po_p', bufs=5) as po_p, \
             tc.tile_pool(name='ps', bufs=1, space='PSUM') as psp:

            # ---- consolidated input DMAs; index tables go via the idle
            # Activation engine's HWDGE so the first SWDGE prep starts early
            it_sb = per.tile([128, ICOLS], i16)
            c00 = PE_CH[0] * 8
            nc.sync.dma_start(out=it_sb[:, 0:c00], in_=itab[:, 0:c00])
            nc.scalar.dma_start(out=it_sb[:, c00:ICOLS], in_=itab[:, c00:ICOLS])
            ft_sb = per.tile([128, FCOLS], f32)
            nc.sync.dma_start(out=ft_sb[:, 0:F_TR], in_=ftab[:, 0:F_TR])
            nc.sync.dma_start(out=ft_sb[:, F_TR:FCOLS], in_=ftab[:, F_TR:FCOLS])
            stm_sb = per.tile([128, NT], bf16)
            nc.scalar.dma_start(out=stm_sb, in_=stmb[:, :])
            nc.gpsimd.load_library(library_config.mlp)

            iota_b = per.tile([128, 128], bf16)
            nc.vector.tensor_copy(iota_b, ft_sb[:, F_IO:F_IO + 128])
            t_bf = per.tile([128, 128], bf16)
            nc.vector.tensor_copy(t_bf, ft_sb[:, F_TR:F_TR + 128])
            ones_b = per.tile([128, 1], bf16)
            nc.vector.memset(ones_b, 1.0)

            # ---- one-hot tiles (singles region: second half all-zero,
            # statically skipped) ----
            hpair = per.tile([128, PEG + POG, 256], bf16)
            for g in range(PEG + POG):
                nc.vector.tensor_scalar(out=hpair[:, g, 0:128], in0=iota_b,
                                        scalar1=ft_sb[:, F_MP1 + g:F_MP1 + g + 1],
                                        scalar2=None, op0=Alu.is_equal)
                if (g % PEG) < PSPLIT:
                    nc.vector.tensor_scalar(out=hpair[:, g, 128:256], in0=iota_b,
                                            scalar1=ft_sb[:, F_MP2 + g:F_MP2 + g + 1],
                                            scalar2=None, op0=Alu.is_equal)
            # ---- gathers + transitions ICs + matmul accumulation ----
            ps_emit = psp.tile([128, 128], f32)
            ps_cnt = psp.tile([128, 1], f32)
            tg_out = per.tile([128, NT], bf16)

            streams = [
                (emp, I_PE, PE_CH, 256, 0, pe_p),
                (emo, I_PO, PO_CH, 256, PEG, po_p),
            ]
            plan = []
            for dram, ibase, chunks, width, gbase, pool in streams:
                g0 = 0
                for ng in chunks:
                    plan.append((dram, ibase, g0, ng, width, gbase, pool))
                    g0 += ng

            n_mm = sum((2 if (g0 + c) < PSPLIT else 1)
                       for _, _, g0, ng, _, _, _ in plan for c in range(ng))
            mm_i = 0
            nh = NT // NIC
            ic_at = len(PE_CH)  # slot ICs into the PO-stream buffer waits
            for ci, (dram, ibase, g0, ng, width, gbase, pool) in enumerate(plan):
                nidx = ng * 128
                gt = pool.tile([128, ng, width], bf16, tag='g')
                nc.gpsimd.dma_gather(
                    out_ap=gt, in_ap=dram[:, :],
                    idxs_ap=it_sb[:, ibase + g0 * 8:ibase + (g0 + ng) * 8],
                    num_idxs=nidx, num_idxs_reg=nidx, elem_size=width)
                # transitions ICs mid-stream: off the tail, and tg_out is
                # ready while DVE still has one-hot slack
                if ic_at <= ci < ic_at + NIC:
                    k = ci - ic_at
                    c0 = I_T + k * nh // 16
                    c1 = I_T + (k + 1) * nh // 16
                    nc.gpsimd.indirect_copy(
                        tg_out[:, k * nh:(k + 1) * nh],
                        t_bf,
                        it_sb[:, c0:c1].bitcast(mybir.dt.uint16), True)

                for c in range(ng):
                    gg = gbase + g0 + c
                    halves = [(hpair[:, gg, 0:128], gt[:, c, 0:128])]
                    if (g0 + c) < PSPLIT:
                        halves.append((hpair[:, gg, 128:256], gt[:, c, 128:256]))
                    for h, rhs in halves:
                        nc.tensor.matmul(ps_emit, lhsT=h, rhs=rhs,
                                         start=(mm_i == 0), stop=(mm_i == n_mm - 1),
                                         skip_group_check=True)
                        nc.tensor.matmul(ps_cnt, lhsT=h, rhs=ones_b,
                                         start=(mm_i == 0), stop=(mm_i == n_mm - 1),
                                         skip_group_check=True)
                        mm_i += 1
            tmul = per.tile([128, NT], bf16)
            red = per.tile([128, 8], f32)
            nc.vector.memset(red, 0.0)
            # fused multiply + free-dim accumulate in one DVE pass
            nc.vector.scalar_tensor_tensor(out=tmul, in0=tg_out, scalar=1.0,
                                           in1=stm_sb, op0=Alu.mult,
                                           op1=Alu.mult,
                                           accum_out=red[:, 1:2])

            # ---- final: diagonal + count -> [128,8] partials ----
            scr = per.tile([128, 128], f32)
            nc.vector.tensor_mul(scr, ps_emit, ft_sb[:, F_ID:F_ID + 128])
            nc.vector.tensor_reduce(out=red[:, 0:1], in_=scr,
                                    axis=mybir.AxisListType.X, op=Alu.add)
            nc.vector.tensor_copy(red[:, 2:3], ps_cnt)
            nc.sync.dma_start(out=out[:, :], in_=red)

    from concourse.library_overlay import lower_extended_insts
    lower_extended_insts(nc)
    return nc


def _wrap16(flat, cols):
    """slot i -> [i%16, i//16], replicated across the 8 gpsimd cores."""
    return np.tile(flat.reshape(cols, 16).T, (8, 1))


def _prep_core(em_bf, tg_flat, mk_flat, lo):
    """Host-side index prep for one core's shard [lo, lo+NPOS)."""
    tg = tg_flat[lo:lo + NPOS]
    mk = mk_flat[lo:lo + NPOS]
    m = mk.astype(np.int8)

    # ---- run decomposition into pair slots (start, t1, t2) ----
    # even runs: pairs (a, a+1)...; odd runs: pairs from the start, single
    # at the run END, gathered as pair (p, p+1) with the masked partner's
    # tag forced out-of-range (p=NPOS-1 flips to (NPOS-2, p), tag1 forced).
    prev = np.empty_like(m)
    prev[0] = 0
    prev[1:] = m[:-1]
    nxt = np.empty_like(m)
    nxt[-1] = 0
    nxt[:-1] = m[1:]
    starts = np.nonzero(m & (1 - prev))[0]
    ends = np.nonzero(m & (1 - nxt))[0]
    lens = ends - starts + 1
    singles = ends[(lens & 1) == 1]
    npair = lens >> 1
    tot = int(npair.sum())
    run_off = np.zeros(len(starts), dtype=np.int64)
    np.cumsum(npair[:-1], out=run_off[1:])
    k = np.arange(tot) - np.repeat(run_off, npair)
    pstart = np.repeat(starts, npair) + 2 * k
    PAD = 1000.0
    p_t1 = tg[pstart].astype(np.float32)
    p_t2 = tg[pstart + 1].astype(np.float32)
    s_last = singles == NPOS - 1
    s_start = np.where(s_last, NPOS - 2, singles)
    s_t1 = np.where(s_last, PAD, tg[np.minimum(singles, NPOS - 1)]
                    .astype(np.float32))
    s_t2 = np.where(s_last, tg[NPOS - 1].astype(np.float32), PAD)

    # region P = real pairs in groups [0, PSPLIT); region S = singles in
    # groups [PSPLIT, PEG) with the t2 one-hot statically skipped.  A
    # "last-position" single needs its t2 half -> force it into region P.
    def build_class(par):
        psel = (pstart & 1) == par
        ssel = ((s_start & 1) == par)
        cp_s = pstart[psel]
        cp_t1, cp_t2 = p_t1[psel], p_t2[psel]
        cs_s = s_start[ssel]
        cs_t1, cs_t2 = s_t1[ssel], s_t2[ssel]
        promote = cs_t2 != PAD  # s_last singles: t2 half required
        cp_s = np.concatenate([cp_s, cs_s[promote]])
        cp_t1 = np.concatenate([cp_t1, cs_t1[promote]])
        cp_t2 = np.concatenate([cp_t2, cs_t2[promote]])
        cs_s, cs_t1 = cs_s[~promote], cs_t1[~promote]
        nP, nS = cp_s.size, cs_s.size
        assert nP <= PSPLIT * 128, f"pairs {nP} > {PSPLIT * 128}"
        assert nS <= (PEG - PSPLIT) * 128, f"singles {nS}"
        starts_full = np.zeros(PEG * 128, dtype=np.int64)
        t1f = np.full(PEG * 128, PAD, dtype=np.float32)
        t2f = np.full(PEG * 128, PAD, dtype=np.float32)
        starts_full[:nP] = cp_s
        t1f[:nP] = cp_t1
        t2f[:nP] = cp_t2
        o = PSPLIT * 128
        starts_full[o:o + nS] = cs_s
        t1f[o:o + nS] = cs_t1
        # pad slots keep start 0 -> valid gather of row 0, zero one-hot
        if par == 1:
            starts_full[(starts_full & 1) == 0] = 1  # pads in odd class
        idx = ((starts_full - par) >> 1).astype(np.int16)
        return idx, t1f, t2f

    pe_idx, pe_t1, pe_t2 = build_class(0)
    po_idx, po_t1, po_t2 = build_class(1)

    def tagtab(vals, cap):
        return vals.reshape(cap, 128).T  # [p, g]

    # ---- transition pairs: within-row (s>=1), both masked ----
    mk2 = mk.reshape(BSH, S)
    tg2 = tg.reshape(BSH, S)
    pm = mk2[:, 1:] & mk2[:, :-1]
    tp = tg2[:, :-1][pm].astype(np.int64)
    tc = tg2[:, 1:][pm].astype(np.int64)
    tidx = np.zeros((128, NTCOL), dtype=np.int16)
    stm = np.zeros((128, NT), dtype=np.float32)
    for g in range(8):
        sel = (tp >> 4) == g
        tpg = tp[sel]
        tcg = tc[sel]
        cnt = tpg.size
        assert cnt <= NT, f"bucket {g} count {cnt} exceeds NT {NT}"
        buf = np.zeros(NT, dtype=np.uint16)
        buf[:cnt] = tcg.astype(np.uint16)
        tidx[16 * g:16 * (g + 1), :] = buf.reshape(NTCOL, 16).T.view(np.int16)
        j = np.arange(cnt)
        stm[tpg, j] = 1.0

    itabv = np.concatenate([
        _wrap16(pe_idx, PEG * 8), _wrap16(po_idx, POG * 8), tidx], axis=1)
    ftabv = np.concatenate([
        tagtab(pe_t1, PEG), tagtab(po_t1, POG),
        tagtab(pe_t2, PEG), tagtab(po_t2, POG),
        np.tile(np.arange(128, dtype=np.float32), (128, 1)),
        np.zeros((128, 128), np.float32),  # tr placeholder, filled by caller
        np.eye(128, dtype=np.float32)], axis=1)

    emc = em_bf[lo:lo + NPOS]
    flat = emc.reshape(-1)
    return {'emp': flat.reshape(NPOS // 2, 256),
            'emo': np.ascontiguousarray(
                flat[128:NPOS * 128 - 128].reshape(NPOS // 2 - 1, 256)),
            'itab': np.ascontiguousarray(itabv),
            'ftab': np.ascontiguousarray(ftabv),
            'stmb': stm.astype(ml_dtypes.bfloat16)}


_nc_cache = None
last_results = None


def kernel(emissions, tags, mask, transitions, _trace=False):
    global _nc_cache, last_results
    from concourse.bass_utils import run_bass_kernel_spmd
    if _nc_cache is None:
        _nc_cache = _build()
    nc = _nc_cache

    em_bf = np.ascontiguousarray(
        emissions.reshape(B * S, T).astype(ml_dtypes.bfloat16))
    tg_flat = tags.reshape(-1).astype(np.int64)
    mk_flat = mask.reshape(-1).astype(bool)
    trf = transitions.astype(np.float32)

    in_maps = []
    for c in range(NCORES):
        mmap = _prep_core(em_bf, tg_flat, mk_flat, c * NPOS)
        mmap['ftab'][:, F_TR:F_TR + 128] = trf
        in_maps.append(mmap)

    res = run_bass_kernel_spmd(nc, in_maps, core_ids=list(range(NCORES)),
                               trace=_trace)
    last_results = res
    emit = trans = cnt = 0.0
    for r in res.results:
        v = r['out']  # [128, 8] partials
        emit += float(v[:, 0].sum())
        trans += float(v[:, 1].sum())
        cnt += float(v[:, 2].sum())
    return np.float32((emit + trans) / cnt)


# revision 14
# speedup vs baseline: 3.1905x; 1.0260x over previous
"""CRF loss kernel for Trainium2 (8 NeuronCores, data-parallel over batch).

v4 = v3 + scheduling fixes:
  - all small inputs arrive in TWO consolidated DMAs (one int16 table,
    one fp32 table) issued before the gathers, so the one-hot builds and
    transition IndirectCopies never stall the Pool/DVE pipelines.
  - IndirectCopies split into 4x320 and slotted between early gather
    descriptor preps (each fits the prep/transfer slack: no DMA bubble).
  - transition multiply+reduce runs mid-stream, off the tail.
  - per-core output is [128,8] partials; host sums partitions & cores.

Core strategy (per core = 64 batch rows, 32768 positions):
  - emissions host-cast to bf16; maximal mask=1 runs fetched by SWDGE
    dma_gather as pairs (512B, line rate) + one single (256B) per odd
    run, via three streams (even pairs / odd pairs / singles).
  - em[pos, tag] selected by one-hot diag-matmul on PE (bf16, fp32
    accum); ones-column matmuls accumulate the mask count.
  - transitions: pm=1 pairs bucketed by tp//16; IndirectCopy reads
    T[p, tc_j] on all 16 partitions of the bucket's gpsimd core; a 0/1
    selector mask keeps partition tp_j; multiply+reduce on DVE.
"""
import sys
import json

for p in ('/opt/trn_rl_repo', '/opt/trn_rl_repo/concourse'):
    if p not in sys.path:
        sys.path.insert(0, p)

import numpy as np
import ml_dtypes

B, S, T = 512, 512, 128
NCORES = 8
BSH = B // NCORES              # 64 batch rows per core
NPOS = BSH * S                 # 32768 positions per core

# v5: singles are merged into the pair streams (the masked partner row is
# gathered too; its one-hot row is zero), so every descriptor is a 512B
# line-rate transfer and there are only two gather streams.
PEG = 46                       # even-start pair-slot groups (46*128=5888)
POG = 46                       # odd-start pair-slot groups
# chunks capped at 7 groups (896 descriptors): >=1280 descriptors in one
# SWDGE gather overflows the 1024-desc scratch ring and wedges the device
PE_CH = [7, 7, 7, 7, 7, 7, 4]
PO_CH = [7, 7, 7, 7, 7, 7, 4]
NT = 1280                      # padded transition pairs per gpsimd core
NTCOL = NT // 16
NIC = 2                        # transitions IndirectCopy split (<=1024 dst)
# int16 table layout: [pidx | oidx | tidx]
I_PE, I_PO, I_T = 0, PEG * 8, (PEG + POG) * 8
ICOLS = I_T + NTCOL
PSPLIT = 23                    # groups 0..22 per class hold real pairs;
                               # groups 23..45 hold singles (t2 half skipped)
# fp32 table layout: [mp1 | mp2 | iota | tr | ident]
F_MP1 = 0
F_MP2 = F_MP1 + PEG + POG
F_IO = F_MP2 + PEG + POG
F_TR = F_IO + 128
F_ID = F_TR + 128
FCOLS = F_ID + 128


def _split_waits_json(bir_bytes: bytes, max_waits: int = 1) -> bytes:
    """This walrus build accepts at most ONE sync-wait per instruction;
    hoist extra waits onto single-wait NoOps inserted before the inst."""
    d = json.loads(bir_bytes)
    ctr = 0
    for f in d['functions']:
        for blk in f['blocks']:
            insts = blk.get('instructions')
            if not insts:
                continue
            out = []
            changed = False
            for ins in insts:
                si = ins.get('sync_info')
                if si and len(si.get('on_wait') or []) > max_waits:
                    waits = si['on_wait']
                    for w in waits[:-max_waits]:
                        ctr += 1
                        nop = {'engine': ins['engine'], 'ins': [], 'outs': [],
                               'name': f'wsplit-{ctr}', 'opcode': 'NoOp',
                               'sync_info': {'on_wait': [w], 'on_update': []}}
                        if 'debug' in ins:
                            nop['debug'] = ins['debug']
                        out.append(nop)
                    si['on_wait'] = waits[-max_waits:]
                    changed = True
                out.append(ins)
            if changed:
                blk['instructions'] = out
    return json.dumps(d).encode()


_patched = False


def _install_patch(bass_module):
    global _patched
    if _patched:
        return
    _patched = True
    orig = bass_module.Bass.to_json_bytes

    def patched(self):
        return _split_waits_json(orig(self))

    bass_module.Bass.to_json_bytes = patched


def _build():
    import concourse.bass as bass
    import concourse.mybir as mybir
    import concourse.tile as tile
    from concourse import library_config
    _install_patch(bass)
    f32 = mybir.dt.float32
    bf16 = mybir.dt.bfloat16
    i16 = mybir.dt.int16
    Alu = mybir.AluOpType

    nc = bass.Bass()
    emp = nc.dram_tensor('emp', [NPOS // 2, 256], bf16, kind='ExternalInput')
    emo = nc.dram_tensor('emo', [NPOS // 2 - 1, 256], bf16, kind='ExternalInput')
    itab = nc.dram_tensor('itab', [128, ICOLS], i16, kind='ExternalInput')
    ftab = nc.dram_tensor('ftab', [128, FCOLS], f32, kind='ExternalInput')
    stmb = nc.dram_tensor('stmb', [128, NT], bf16, kind='ExternalInput')
    out = nc.dram_tensor('out', [128, 8], f32, kind='ExternalOutput')

    with tile.TileContext(nc) as tc:
        with tc.tile_pool(name='per', bufs=1) as per, \
             tc.tile_pool(name='pe_p', bufs=5) as pe_p, \
             tc.tile_pool(name='po_p', bufs=5) as po_p, \
             tc.tile_pool(name='ps', bufs=1, space='PSUM') as psp:

            # ---- consolidated input DMAs; index tables go via the idle
            # Activation engine's HWDGE so the first SWDGE prep starts early
            it_sb = per.tile([128, ICOLS], i16)
            c00 = I_PO  # all even-class idx cols first: ungates PE preps
            nc.sync.dma_start(out=it_sb[:, 0:c00], in_=itab[:, 0:c00])
            nc.scalar.dma_start(out=it_sb[:, c00:ICOLS], in_=itab[:, c00:ICOLS])
            ft_sb = per.tile([128, FCOLS], f32)
            nc.sync.dma_start(out=ft_sb[:, 0:F_TR], in_=ftab[:, 0:F_TR])
            nc.sync.dma_start(out=ft_sb[:, F_TR:FCOLS], in_=ftab[:, F_TR:FCOLS])
            stm_sb = per.tile([128, NT], bf16)
            nc.scalar.dma_start(out=stm_sb, in_=stmb[:, :])
            nc.gpsimd.load_library(library_config.mlp)

            iota_b = per.tile([128, 128], bf16)
            nc.vector.tensor_copy(iota_b, ft_sb[:, F_IO:F_IO + 128])
            t_bf = per.tile([128, 128], bf16)
            nc.vector.tensor_copy(t_bf, ft_sb[:, F_TR:F_TR + 128])
            ones_b = per.tile([128, 1], bf16)
            nc.vector.memset(ones_b, 1.0)

            # ---- one-hot tiles (singles region: second half all-zero,
            # statically skipped) ----
            hpair = per.tile([128, PEG + POG, 256], bf16)
            for g in range(PEG + POG):
                nc.vector.tensor_scalar(out=hpair[:, g, 0:128], in0=iota_b,
                                        scalar1=ft_sb[:, F_MP1 + g:F_MP1 + g + 1],
                                        scalar2=None, op0=Alu.is_equal)
                if (g % PEG) < PSPLIT:
                    nc.vector.tensor_scalar(out=hpair[:, g, 128:256], in0=iota_b,
                                            scalar1=ft_sb[:, F_MP2 + g:F_MP2 + g + 1],
                                            scalar2=None, op0=Alu.is_equal)
            # ---- gathers + transitions ICs + matmul accumulation ----
            ps_emit = psp.tile([128, 128], f32)
            ps_cnt = psp.tile([128, 1], f32)
            tg_out = per.tile([128, NT], bf16)

            streams = [
                (emp, I_PE, PE_CH, 256, 0, pe_p),
                (emo, I_PO, PO_CH, 256, PEG, po_p),
            ]
            plan = []
            for dram, ibase, chunks, width, gbase, pool in streams:
                g0 = 0
                for ng in chunks:
                    plan.append((dram, ibase, g0, ng, width, gbase, pool))
                    g0 += ng

            n_mm = sum((2 if (g0 + c) < PSPLIT else 1)
                       for _, _, g0, ng, _, _, _ in plan for c in range(ng))
            mm_i = 0
            nh = NT // NIC
            ic_at = len(PE_CH)  # slot ICs into the PO-stream buffer waits
            for ci, (dram, ibase, g0, ng, width, gbase, pool) in enumerate(plan):
                nidx = ng * 128
                gt = pool.tile([128, ng, width], bf16, tag='g')
                nc.gpsimd.dma_gather(
                    out_ap=gt, in_ap=dram[:, :],
                    idxs_ap=it_sb[:, ibase + g0 * 8:ibase + (g0 + ng) * 8],
                    num_idxs=nidx, num_idxs_reg=nidx, elem_size=width)
                # transitions ICs after the last prep: they overlap the
                # final transfers, and the fused transitions reduce then
                # overlaps the PE/diag tail
                if ci == len(plan) - 1:
                    for k in range(NIC):
                        c0 = I_T + k * nh // 16
                        c1 = I_T + (k + 1) * nh // 16
                        nc.gpsimd.indirect_copy(
                            tg_out[:, k * nh:(k + 1) * nh],
                            t_bf,
                            it_sb[:, c0:c1].bitcast(mybir.dt.uint16), True)

                for c in range(ng):
                    gg = gbase + g0 + c
                    halves = [(hpair[:, gg, 0:128], gt[:, c, 0:128])]
                    if (g0 + c) < PSPLIT:
                        halves.append((hpair[:, gg, 128:256], gt[:, c, 128:256]))
                    for h, rhs in halves:
                        nc.tensor.matmul(ps_emit, lhsT=h, rhs=rhs,
                                         start=(mm_i == 0), stop=(mm_i == n_mm - 1),
                                         skip_group_check=True)
                        nc.tensor.matmul(ps_cnt, lhsT=h, rhs=ones_b,
                                         start=(mm_i == 0), stop=(mm_i == n_mm - 1),
                                         skip_group_check=True)
                        mm_i += 1
            tmul = per.tile([128, NT], bf16)
            red = per.tile([128, 8], f32)
            nc.vector.memset(red, 0.0)
            # fused multiply + free-dim accumulate in one DVE pass
            nc.vector.scalar_tensor_tensor(out=tmul, in0=tg_out, scalar=1.0,
                                           in1=stm_sb, op0=Alu.mult,
                                           op1=Alu.mult,
                                           accum_out=red[:, 1:2])

            # ---- final: diagonal + count -> [128,8] partials ----
            scr = per.tile([128, 128], f32)
            # fused diag-mask multiply + free-dim accumulate (one DVE pass)
            nc.vector.scalar_tensor_tensor(out=scr, in0=ps_emit, scalar=1.0,
                                           in1=ft_sb[:, F_ID:F_ID + 128],
                                           op0=Alu.mult, op1=Alu.mult,
                                           accum_out=red[:, 0:1])
            nc.vector.tensor_copy(red[:, 2:3], ps_cnt)
            nc.sync.dma_start(out=out[:, :], in_=red)

    from concourse.library_overlay import lower_extended_insts
    lower_extended_insts(nc)
    return nc


def _wrap16(flat, cols):
    """slot i -> [i%16, i//16], replicated across the 8 gpsimd cores."""
    return np.tile(flat.reshape(cols, 16).T, (8, 1))


def _prep_core(em_bf, tg_flat, mk_flat, lo):
    """Host-side index prep for one core's shard [lo, lo+NPOS)."""
    tg = tg_flat[lo:lo + NPOS]
    mk = mk_flat[lo:lo + NPOS]
    m = mk.astype(np.int8)

    # ---- run decomposition into pair slots (start, t1, t2) ----
    # even runs: pairs (a, a+1)...; odd runs: pairs from the start, single
    # at the run END, gathered as pair (p, p+1) with the masked partner's
    # tag forced out-of-range (p=NPOS-1 flips to (NPOS-2, p), tag1 forced).
    prev = np.empty_like(m)
    prev[0] = 0
    prev[1:] = m[:-1]
    nxt = np.empty_like(m)
    nxt[-1] = 0
    nxt[:-1] = m[1:]
    starts = np.nonzero(m & (1 - prev))[0]
    ends = np.nonzero(m & (1 - nxt))[0]
    lens = ends - starts + 1
    singles = ends[(lens & 1) == 1]
    npair = lens >> 1
    tot = int(npair.sum())
    run_off = np.zeros(len(starts), dtype=np.int64)
    np.cumsum(npair[:-1], out=run_off[1:])
    k = np.arange(tot) - np.repeat(run_off, npair)
    pstart = np.repeat(starts, npair) + 2 * k
    PAD = 1000.0
    p_t1 = tg[pstart].astype(np.float32)
    p_t2 = tg[pstart + 1].astype(np.float32)
    s_last = singles == NPOS - 1
    s_start = np.where(s_last, NPOS - 2, singles)
    s_t1 = np.where(s_last, PAD, tg[np.minimum(singles, NPOS - 1)]
                    .astype(np.float32))
    s_t2 = np.where(s_last, tg[NPOS - 1].astype(np.float32), PAD)

    # region P = real pairs in groups [0, PSPLIT); region S = singles in
    # groups [PSPLIT, PEG) with the t2 one-hot statically skipped.  A
    # "last-position" single needs its t2 half -> force it into region P.
    def build_class(par):
        psel = (pstart & 1) == par
        ssel = ((s_start & 1) == par)
        cp_s = pstart[psel]
        cp_t1, cp_t2 = p_t1[psel], p_t2[psel]
        cs_s = s_start[ssel]
        cs_t1, cs_t2 = s_t1[ssel], s_t2[ssel]
        promote = cs_t2 != PAD  # s_last singles: t2 half required
        cp_s = np.concatenate([cp_s, cs_s[promote]])
        cp_t1 = np.concatenate([cp_t1, cs_t1[promote]])
        cp_t2 = np.concatenate([cp_t2, cs_t2[promote]])
        cs_s, cs_t1 = cs_s[~promote], cs_t1[~promote]
        nP, nS = cp_s.size, cs_s.size
        assert nP <= PSPLIT * 128, f"pairs {nP} > {PSPLIT * 128}"
        assert nS <= (PEG - PSPLIT) * 128, f"singles {nS}"
        starts_full = np.zeros(PEG * 128, dtype=np.int64)
        t1f = np.full(PEG * 128, PAD, dtype=np.float32)
        t2f = np.full(PEG * 128, PAD, dtype=np.float32)
        starts_full[:nP] = cp_s
        t1f[:nP] = cp_t1
        t2f[:nP] = cp_t2
        o = PSPLIT * 128
        starts_full[o:o + nS] = cs_s
        t1f[o:o + nS] = cs_t1
        # pad slots keep start 0 -> valid gather of row 0, zero one-hot
        if par == 1:
            starts_full[(starts_full & 1) == 0] = 1  # pads in odd class
        idx = ((starts_full - par) >> 1).astype(np.int16)
        return idx, t1f, t2f

    pe_idx, pe_t1, pe_t2 = build_class(0)
    po_idx, po_t1, po_t2 = build_class(1)

    def tagtab(vals, cap):
        return vals.reshape(cap, 128).T  # [p, g]

    # ---- transition pairs: within-row (s>=1), both masked ----
    mk2 = mk.reshape(BSH, S)
    tg2 = tg.reshape(BSH, S)
    pm = mk2[:, 1:] & mk2[:, :-1]
    tp = tg2[:, :-1][pm].astype(np.int64)
    tc = tg2[:, 1:][pm].astype(np.int64)
    tidx = np.zeros((128, NTCOL), dtype=np.int16)
    stm = np.zeros((128, NT), dtype=np.float32)
    for g in range(8):
        sel = (tp >> 4) == g
        tpg = tp[sel]
        tcg = tc[sel]
        cnt = tpg.size
        assert cnt <= NT, f"bucket {g} count {cnt} exceeds NT {NT}"
        buf = np.zeros(NT, dtype=np.uint16)
        buf[:cnt] = tcg.astype(np.uint16)
        tidx[16 * g:16 * (g + 1), :] = buf.reshape(NTCOL, 16).T.view(np.int16)
        j = np.arange(cnt)
        stm[tpg, j] = 1.0

    itabv = np.concatenate([
        _wrap16(pe_idx, PEG * 8), _wrap16(po_idx, POG * 8), tidx], axis=1)
    ftabv = np.concatenate([
        tagtab(pe_t1, PEG), tagtab(po_t1, POG),
        tagtab(pe_t2, PEG), tagtab(po_t2, POG),
        np.tile(np.arange(128, dtype=np.float32), (128, 1)),
        np.zeros((128, 128), np.float32),  # tr placeholder, filled by caller
        np.eye(128, dtype=np.float32)], axis=1)

    emc = em_bf[lo:lo + NPOS]
    flat = emc.reshape(-1)
    return {'emp': flat.reshape(NPOS // 2, 256),
            'emo': np.ascontiguousarray(
                flat[128:NPOS * 128 - 128].reshape(NPOS // 2 - 1, 256)),
            'itab': np.ascontiguousarray(itabv),
            'ftab': np.ascontiguousarray(ftabv),
            'stmb': stm.astype(ml_dtypes.bfloat16)}


_nc_cache = None
last_results = None


def kernel(emissions, tags, mask, transitions, _trace=False):
    global _nc_cache, last_results
    from concourse.bass_utils import run_bass_kernel_spmd
    if _nc_cache is None:
        _nc_cache = _build()
    nc = _nc_cache

    em_bf = np.ascontiguousarray(
        emissions.reshape(B * S, T).astype(ml_dtypes.bfloat16))
    tg_flat = tags.reshape(-1).astype(np.int64)
    mk_flat = mask.reshape(-1).astype(bool)
    trf = transitions.astype(np.float32)

    in_maps = []
    for c in range(NCORES):
        mmap = _prep_core(em_bf, tg_flat, mk_flat, c * NPOS)
        mmap['ftab'][:, F_TR:F_TR + 128] = trf
        in_maps.append(mmap)

    res = run_bass_kernel_spmd(nc, in_maps, core_ids=list(range(NCORES)),
                               trace=_trace)
    last_results = res
    emit = trans = cnt = 0.0
    for r in res.results:
        v = r['out']  # [128, 8] partials
        emit += float(v[:, 0].sum())
        trans += float(v[:, 1].sum())
        cnt += float(v[:, 2].sum())
    return np.float32((emit + trans) / cnt)
